# revision 1
# baseline (speedup 1.0000x reference)
"""Trainium2 Bass kernel for a BERT decoder layer (no-memory-untied variant).

Distribution: 8 NeuronCores. Core c handles batch element b=c//2 and
sequence-column half r=c%2 (columns [512r:512r+512] of both the decoder and
encoder sequences).  K/V projections are computed over the full sequence on
both cores of a pair (duplicated: +2.15 GFLOP/core, cheaper and far less
risky than the ~210 us modeled pair-AllGather); everything else (Q
projections, attention, layernorms, output dense) is column-local, so the
kernel has no communication at all.

Layouts: activations are kept feature-major [D, L] on chip ([128, 8, 512]
tiles: partition = d%128, d-tile = d//128, column = local l).  V is produced
sequence-major [L, D] directly by using the activation tile as the stationary
matmul operand.  Attention scores are computed transposed (sT[k, q]) so that
softmax normalization can be folded into the ctx matmul via an appended
ones-column ([V|1]), with the final 1/Z applied to the 64-row ctx output.
Softmax is computed without max-subtraction (scores are O(1) here), matching
jax.nn.softmax to fp32 rounding.  Heads are processed in pairs occupying
PE row-groups 0-63 / 64-127 so the K=64 score matmuls overlap.

Dtypes: projections run on the PE in float32r (tf32-like); attention
internals (Q/K/V tiles and exp outputs) are bf16; PSUM accumulation and the
residual/LN chain are fp32/f32r.
"""
import sys

sys.path.insert(0, '/opt/trn_rl_repo')

import contextlib

import numpy as np
import ml_dtypes

import concourse.bass as bass
from concourse import bacc
import concourse.tile as tile
from concourse import mybir
from concourse.bass_utils import run_bass_kernel_spmd

BF = mybir.dt.bfloat16
F32 = mybir.dt.float32
F32R = mybir.dt.float32r
EXP = mybir.ActivationFunctionType.Exp
LN_ = mybir.ActivationFunctionType.Ln

N, LT, D, H, HD = 4, 1024, 1024, 16, 64
EPS = 1e-12
P = 128
NT = D // P          # 8 d-tiles
W = 512              # per-core column count
KT = LT // P         # 8 k-tiles (full sequence)
SCALE = float(1.0 / np.sqrt(HD))

_CACHE = {}
LAST_RESULT = None


# --------------------------------------------------------------------------
# device kernel construction
# --------------------------------------------------------------------------

def _build_nc(with_ln_wb, with_bias):
    nc = bacc.Bacc("TRN2", target_bir_lowering=False, debug=False,
                   num_devices=8)

    # ---- I/O ----
    x_in = nc.declare_dram_parameter("x", [P, NT, W], F32R, isOutput=False)
    xbf_in = nc.declare_dram_parameter("xbf", [P, NT, LT], BF, isOutput=False)
    ebf_in = nc.declare_dram_parameter("encbf", [P, NT, LT], BF,
                                       isOutput=False)
    mk_in = nc.declare_dram_parameter("masks", [P, 64], BF,
                                      isOutput=False)
    wts = {}
    for nm in ["wq", "wqc", "wo"]:
        wts[nm] = nc.declare_dram_parameter(nm, [NT, P, NT, P], F32R,
                                            isOutput=False)
    for nm in ["wk", "wkc"]:
        wts[nm] = nc.declare_dram_parameter(nm, [NT, P, NT, P], BF,
                                            isOutput=False)
    for nm in ["wv", "wvc"]:
        wts[nm] = nc.declare_dram_parameter(nm, [2, P, NT, W], BF,
                                            isOutput=False)
    if with_ln_wb:
        lnp = nc.declare_dram_parameter("lnp", [6, P, NT], F32, isOutput=False)
    if with_bias:
        bia = nc.declare_dram_parameter("bias", [7, P, NT], F32, isOutput=False)
        vbf = nc.declare_dram_parameter("vbflat", [2, 1, D], F32,
                                        isOutput=False)
    y_out = nc.declare_dram_parameter("y", [P, NT, W], F32, isOutput=True)


    with tile.TileContext(nc) as tc:
        ctx = contextlib.ExitStack()
        with ctx:
            pool = ctx.enter_context(tc.tile_pool(name="main", bufs=1))
            wpool = ctx.enter_context(tc.tile_pool(name="w", bufs=3))
            wvpool = ctx.enter_context(tc.tile_pool(name="wv", bufs=1))
            epool = ctx.enter_context(tc.tile_pool(name="e", bufs=6))
            cspool = ctx.enter_context(tc.tile_pool(name="cs", bufs=2))
            bcpool = ctx.enter_context(tc.tile_pool(name="bc", bufs=1))
            bczpool = ctx.enter_context(tc.tile_pool(name="bcz", bufs=2))
            smpool = ctx.enter_context(tc.tile_pool(name="sm", bufs=2))
            statpool = ctx.enter_context(tc.tile_pool(name="stat", bufs=1))
            ps_s = ctx.enter_context(
                tc.tile_pool(name="ps_s", bufs=2, space="PSUM"))
            ps_c = ctx.enter_context(
                tc.tile_pool(name="ps_c", bufs=1, space="PSUM"))
            ps_p = ctx.enter_context(
                tc.tile_pool(name="ps_p", bufs=2, space="PSUM"))

            # ---- constants / small inputs ----
            consts = pool.tile([P, 2], F32, tag="consts")
            nc.vector.memset(consts[:, 0:1], 1.0)
            nc.vector.memset(consts[:, 1:2], EPS)
            ones_r = consts[:, 0:1].bitcast(F32R)
            eps_ap = consts[0:1, 1:2]


            if with_ln_wb:
                lnt = pool.tile([6, P, NT], F32, tag="lnp")
                nc.sync.dma_start(lnt[:], lnp[:])
            if with_bias:
                bt = pool.tile([7, P, NT], F32, tag="bias")
                nc.sync.dma_start(bt[:], bia[:])
                vbt = pool.tile([1, 2, D], F32, tag="vbias")
                nc.sync.dma_start(vbt[:], vbf.rearrange("a b c -> b a c"))

            xbf = pool.tile([P, NT, LT], BF, tag="xbf")
            nc.sync.dma_start(xbf[:], xbf_in[:])
            x32 = pool.tile([P, NT, W], F32R, tag="actA")
            nc.sync.dma_start(x32[:], x_in[:])
            masks = pool.tile([P, 64], BF, tag="masks")
            nc.sync.dma_start(masks[:], mk_in[:])

            # ---------- helpers ----------
            def proj_k_full(w_dram, src_bf, dst, bias_idx):
                """K projection over the full sequence: dst [P, NT, LT] bf16.

                src_bf [P, NT, LT] bf16; weights bf16."""
                for dot in range(NT):
                    wt = wpool.tile([P, NT, P], BF, tag="wb")
                    nc.sync.dma_start(wt[:], w_dram[dot])
                    for blk in range(2):
                        ps = ps_p.tile([P, W], F32, tag="pp")
                        for dit in range(NT):
                            nc.tensor.matmul(
                                ps[:], wt[:, dit, :],
                                src_bf[:, dit, blk * W:(blk + 1) * W],
                                start=(dit == 0), stop=(dit == NT - 1))
                        if with_bias and bias_idx is not None:
                            nc.vector.tensor_scalar_add(
                                dst[:, dot, blk * W:(blk + 1) * W], ps[:],
                                bt[bias_idx, :, dot:dot + 1])
                        elif (dot + blk) % 2 == 0:
                            nc.vector.tensor_copy(
                                dst[:, dot, blk * W:(blk + 1) * W], ps[:])
                        else:
                            nc.scalar.copy(
                                dst[:, dot, blk * W:(blk + 1) * W], ps[:])

            def proj_v_full(wv_dram, src_bf, dst, vb_idx):
                """V projection over the full sequence: dst [P, KT, 16, 65]."""
                for blk in range(2):
                    wt = wvpool.tile([P, NT, W], BF, tag="wv")
                    nc.sync.dma_start(wt[:], wv_dram[blk])
                    for lt in range(KT):
                        ps = ps_p.tile([P, W], F32, tag="pp")
                        for dit in range(NT):
                            nc.tensor.matmul(
                                ps[:], src_bf[:, dit, bass.ts(lt, P)],
                                wt[:, dit, :],
                                start=(dit == 0), stop=(dit == NT - 1))
                        dstv = dst[:, lt, 8 * blk:8 * (blk + 1), 0:64]
                        if with_bias:
                            bb = bcpool.tile([P, W], F32, tag="ub")
                            nc.gpsimd.partition_broadcast(
                                bb[:], vbt[0:1, vb_idx,
                                           blk * W:(blk + 1) * W])
                            nc.vector.tensor_add(
                                dstv, ps[:].rearrange("p (h c) -> p h c",
                                                      c=64), bb[:].rearrange(
                                    "p (h c) -> p h c", c=64))
                        elif (blk + lt) % 2 == 0:
                            nc.vector.tensor_copy(
                                dstv, ps[:].rearrange("p (h c) -> p h c",
                                                      c=64))
                        else:
                            nc.scalar.copy(
                                dstv, ps[:].rearrange("p (h c) -> p h c",
                                                      c=64))

            def proj_feat(w_dram, src, dst, bias_idx):
                """Feature-major projection dst[dout,l]: dst bf16/f32 tile."""
                for dot in range(NT):
                    wt = wpool.tile([P, NT, P], F32R, tag="w")
                    nc.sync.dma_start(wt[:], w_dram[dot])
                    ps = ps_p.tile([P, W], F32, tag="pp")
                    for dit in range(NT):
                        nc.tensor.matmul(ps[:], wt[:, dit, :], src[:, dit, :],
                                         start=(dit == 0), stop=(dit == NT - 1))
                    if with_bias and bias_idx is not None:
                        nc.vector.tensor_scalar_add(
                            dst[:, dot, :], ps[:],
                            bt[bias_idx, :, dot:dot + 1])
                    else:
                        nc.vector.tensor_copy(dst[:, dot, :], ps[:])

            def proj_seq(wv_dram, src, dst, vb_idx):
                """Seq-major V projection into dst [P, 4, 16, 65] (own half).

                Column 64 of each 65-block is left for the ones column."""
                for blk in range(2):
                    wt = wvpool.tile([P, NT, W], F32R, tag="wv")
                    nc.sync.dma_start(wt[:], wv_dram[blk])
                    for lt in range(4):
                        ps = ps_p.tile([P, W], F32, tag="pp")
                        for dit in range(NT):
                            nc.tensor.matmul(
                                ps[:], src[:, dit, bass.ts(lt, P)],
                                wt[:, dit, :],
                                start=(dit == 0), stop=(dit == NT - 1))
                        dstv = dst[:, lt, 8 * blk:8 * (blk + 1), 0:64]
                        if with_bias:
                            bb = bcpool.tile([P, W], F32, tag="ub")
                            nc.gpsimd.partition_broadcast(
                                bb[:], vbt[0:1, vb_idx,
                                           blk * W:(blk + 1) * W])
                            nc.vector.tensor_add(
                                dstv, ps[:].rearrange("p (h c) -> p h c",
                                                      c=64), bb[:].rearrange(
                                    "p (h c) -> p h c", c=64))
                        elif (blk + lt) % 2 == 0:
                            nc.vector.tensor_copy(
                                dstv, ps[:].rearrange("p (h c) -> p h c",
                                                      c=64))
                        else:
                            nc.scalar.copy(
                                dstv, ps[:].rearrange("p (h c) -> p h c",
                                                      c=64))

            def attention_cross(qt, kt, vt, out_tile, masked=False):
                """sT-layout attention (no mask), 512-wide q, head pairs."""
                for hp in range(H // 2):
                    h0, h1 = 2 * hp, 2 * hp + 1
                    dt = hp
                    cps0 = ps_c.tile([P, W], F32, tag="cps0")
                    cps1 = ps_c.tile([P, W], F32, tag="cps1")

                    def _ctx_mms(e, t):
                        nc.tensor.matmul(
                            cps0[0:65, :], vt[:, t, h0, :], e[:, 0, :],
                            start=(t == 0), stop=(t == KT - 1))
                        nc.tensor.matmul(
                            cps1[0:65, :], vt[:, t, h1, :], e[:, 1, :],
                            start=(t == 0), stop=(t == KT - 1))

                    pending = None
                    for t in range(KT):
                        sc = ps_s.tile([P, 2, W], F32, tag="sc")
                        nc.tensor.matmul(
                            sc[:, 0, :], kt[0:64, dt, bass.ts(t, P)],
                            qt[0:64, dt, :], start=True, stop=True)
                        nc.tensor.matmul(
                            sc[:, 1, :], kt[64:P, dt, bass.ts(t, P)],
                            qt[64:P, dt, :], start=True, stop=True)
                        e = epool.tile([P, 2, W], BF, tag="e")
                        nc.scalar.activation(e[:], sc[:], EXP, scale=SCALE)
                        if masked:
                            nc.vector.tensor_mul(
                                e[:], e[:],
                                masks[:, t:t + 1, :].to_broadcast((P, 2, W)))
                        if pending is not None:
                            _ctx_mms(*pending)
                        pending = (e, t)
                    _ctx_mms(*pending)

                    # evacuate both ctx tiles immediately (DVE + ACT in
                    # parallel) so the PSUM banks free for the next pair
                    cs = cspool.tile([65, 2, W], F32, tag="cs")
                    nc.vector.tensor_copy(cs[:, 0, :], cps0[0:65, :])
                    nc.vector.tensor_copy(cs[:, 1, :], cps1[0:65, :])
                    invz = smpool.tile([1, 2, W], F32, tag="invz")
                    nc.vector.reciprocal(invz[0:1, 0, :], cs[64:65, 0, :])
                    nc.vector.reciprocal(invz[0:1, 1, :], cs[64:65, 1, :])
                    izb = bczpool.tile([64, 2, W], F32, tag="izb")
                    nc.gpsimd.partition_broadcast(izb[:], invz[:])
                    nc.vector.tensor_mul(out_tile[0:64, dt, :],
                                         cs[0:64, 0, :], izb[:, 0, :])
                    nc.vector.tensor_mul(out_tile[64:P, dt, :],
                                         cs[0:64, 1, :], izb[:, 1, :])

            def attention_self(qt, kt, vt, out_tile):
                """Uniform causal self-attention (no control flow).

                Fixed-shape score/e tiles; only slices 0..nk-1 are used."""
                for hp in range(H // 2):
                    h0, h1 = 2 * hp, 2 * hp + 1
                    dt = hp
                    csw = cspool.tile([65, 2, 8, 64], F32, tag="cs")
                    for qb in range(8):
                        nk = qb + 1
                        qs = slice(64 * qb, 64 * qb + 64)
                        cps0 = ps_c.tile([P, W], F32, tag="cps0")
                        cps1 = ps_c.tile([P, W], F32, tag="cps1")
                        sc = ps_s.tile([P, 2, KT, 64], F32, tag="sc")
                        for t in range(nk):
                            nc.tensor.matmul(
                                sc[:, 0, t, :], kt[0:64, dt, bass.ts(t, P)],
                                qt[0:64, dt, qs], start=True, stop=True)
                            nc.tensor.matmul(
                                sc[:, 1, t, :], kt[64:P, dt, bass.ts(t, P)],
                                qt[64:P, dt, qs], start=True, stop=True)
                        e = epool.tile([P, 2, KT, 64], BF, tag="e")
                        nc.scalar.activation(e[:, :, 0:nk, :],
                                             sc[:, :, 0:nk, :], EXP,
                                             scale=SCALE)
                        nc.vector.tensor_mul(
                            e[:, :, qb, :], e[:, :, qb, :],
                            masks[:, None, :].to_broadcast((P, 2, 64)))
                        for t in range(nk):
                            nc.tensor.matmul(
                                cps0[0:65, 0:64], vt[:, t, h0, :],
                                e[:, 0, t, :],
                                start=(t == 0), stop=(t == nk - 1))
                            nc.tensor.matmul(
                                cps1[0:65, 0:64], vt[:, t, h1, :],
                                e[:, 1, t, :],
                                start=(t == 0), stop=(t == nk - 1))
                        nc.vector.tensor_copy(csw[:, 0, qb, :],
                                              cps0[0:65, 0:64])
                        nc.vector.tensor_copy(csw[:, 1, qb, :],
                                              cps1[0:65, 0:64])
                    # batched softmax tail: one recip/bcast/mul per head
                    invz = smpool.tile([1, 2, 8, 64], F32, tag="invz")
                    nc.vector.reciprocal(invz[0:1, 0], csw[64:65, 0])
                    nc.vector.reciprocal(invz[0:1, 1], csw[64:65, 1])
                    izb = bczpool.tile([64, 2, 8, 64], F32, tag="izb")
                    nc.gpsimd.partition_broadcast(izb[:], invz[:])
                    nc.vector.tensor_mul(out_tile[0:64, dt, :],
                                         csw[0:64, 0].rearrange(
                                             "p a b -> p (a b)"),
                                         izb[:, 0].rearrange(
                                             "p a b -> p (a b)"))
                    nc.vector.tensor_mul(out_tile[64:P, dt, :],
                                         csw[0:64, 1].rearrange(
                                             "p a b -> p (a b)"),
                                         izb[:, 1].rearrange(
                                             "p a b -> p (a b)"))

            def layernorm(z, out, ln_idx):
                """LN over the d axis of z [P, NT, W] (f32r)."""
                sq = pool.tile([P, NT, W], F32R, tag="scratch")
                for dt in range(NT):
                    nc.scalar.square(sq[:, dt, :], z[:, dt, :])
                s1 = ps_p.tile([P, W], F32, tag="pp")
                s2 = ps_p.tile([P, W], F32, tag="pp")
                for dt in range(NT):
                    nc.tensor.matmul(s1[0:1, :], ones_r, z[:, dt, :],
                                     start=(dt == 0), stop=(dt == NT - 1))
                for dt in range(NT):
                    nc.tensor.matmul(s2[0:1, :], ones_r, sq[:, dt, :],
                                     start=(dt == 0), stop=(dt == NT - 1))
                u = statpool.tile([1, W], F32, tag="u")
                nc.vector.tensor_scalar_mul(u[:], s1[0:1, :], 1.0 / D)
                m2 = statpool.tile([1, W], F32, tag="m2")
                nc.vector.tensor_scalar_mul(m2[:], s2[0:1, :], 1.0 / D)
                uu = statpool.tile([1, W], F32, tag="uu")
                nc.vector.tensor_mul(uu[:], u[:], u[:])
                var = statpool.tile([1, W], F32, tag="var")
                nc.vector.tensor_sub(var[:], m2[:], uu[:])
                lnv = statpool.tile([1, W], F32, tag="lnv")
                nc.scalar.activation(lnv[:], var[:], LN_, bias=eps_ap)
                isd = statpool.tile([1, W], F32, tag="isd")
                nc.scalar.activation(isd[:], lnv[:], EXP, scale=-0.5)
                ub = bcpool.tile([P, W], F32, tag="ub")
                nc.gpsimd.partition_broadcast(ub[:], u[:])
                sb = bcpool.tile([P, W], F32, tag="sb")
                nc.gpsimd.partition_broadcast(sb[:], isd[:])
                for dt in range(NT):
                    nc.vector.tensor_sub(sq[:, dt, :], z[:, dt, :], ub[:])
                    nc.vector.tensor_mul(out[:, dt, :], sq[:, dt, :], sb[:])
                    if with_ln_wb:
                        nc.vector.tensor_scalar(
                            out[:, dt, :], out[:, dt, :],
                            lnt[2 * ln_idx, :, dt:dt + 1],
                            lnt[2 * ln_idx + 1, :, dt:dt + 1],
                            op0=mybir.AluOpType.mult,
                            op1=mybir.AluOpType.add)

            # ---------- phase 1: self K/V projections (full sequence) ----
            ktf = pool.tile([P, NT, LT], BF, tag="ktf")
            proj_k_full(wts["wk"], xbf, ktf, 1)
            vtf = pool.tile([P, KT, 16, 65], BF, tag="vtf")
            nc.vector.memset(vtf[:, :, :, 64:65], 1.0)
            proj_v_full(wts["wv"], xbf, vtf, 0)

            # ---------- phase 2: Q projection ----------
            qt = pool.tile([P, NT, W], BF, tag="qt")
            proj_feat(wts["wq"], x32, qt, 0)

            # ---------- phase 5: self attention (mask-driven causal) ------
            az = pool.tile([P, NT, W], F32, tag="scratch")
            attention_self(qt, ktf, vtf, az)

            # ---------- phase 6: residual + LN1 ----------
            z1 = pool.tile([P, NT, W], F32R, tag="zres")
            for dt in range(NT):
                nc.vector.tensor_add(z1[:, dt, :], az[:, dt, :], x32[:, dt, :])
            a32 = pool.tile([P, NT, W], F32R, tag="a32")
            layernorm(z1, a32, 0)

            # ---------- phase 7: enc K/V projections + cross Q ----------
            encbf = pool.tile([P, NT, LT], BF, tag="xbf")
            nc.sync.dma_start(encbf[:], ebf_in[:])
            ktfe = pool.tile([P, NT, LT], BF, tag="ktf")
            proj_k_full(wts["wkc"], encbf, ktfe, 5)
            vtfe = pool.tile([P, KT, 16, 65], BF, tag="vtf")
            nc.vector.memset(vtfe[:, :, :, 64:65], 1.0)
            proj_v_full(wts["wvc"], encbf, vtfe, 1)
            qtc = pool.tile([P, NT, W], BF, tag="qt")
            proj_feat(wts["wqc"], a32, qtc, 4)

            # ---------- phase 8: cross attention ----------
            cz = pool.tile([P, NT, W], F32, tag="scratch")
            attention_cross(qtc, ktfe, vtfe, cz)

            # ---------- phase 9: residual + LN2 ----------
            z2 = pool.tile([P, NT, W], F32R, tag="zres")
            for dt in range(NT):
                nc.vector.tensor_add(z2[:, dt, :], cz[:, dt, :], a32[:, dt, :])
            c32 = pool.tile([P, NT, W], F32R, tag="actA")
            layernorm(z2, c32, 1)

            # ---------- phase 10: output dense + LN3 ----------
            h32 = pool.tile([P, NT, W], F32, tag="a32")
            proj_feat(wts["wo"], c32, h32, 3)
            z3 = pool.tile([P, NT, W], F32R, tag="zres")
            for dt in range(NT):
                nc.vector.tensor_add(z3[:, dt, :], h32[:, dt, :], c32[:, dt, :])
            layernorm(z3, z3, 2)
            nc.sync.dma_start(y_out[:], z3[:].bitcast(F32))

    nc.compile()
    return nc


# --------------------------------------------------------------------------
# host-side packing
# --------------------------------------------------------------------------

def _feat_pack(xT_cols):
    """[D, W] feature-major -> [128, NT, W] contiguous f32."""
    return np.ascontiguousarray(
        xT_cols.reshape(NT, P, W).transpose(1, 0, 2)).astype(np.float32)


def _w_pack(w):
    """torch-Linear weight [dout, din] -> [NT, P, NT, P] (wT blocked)."""
    wT = np.asarray(w).T  # [din, dout]
    return np.ascontiguousarray(
        wT.reshape(NT, P, NT, P).transpose(2, 1, 0, 3)).astype(np.float32)


def _wv_pack(w):
    """V weight [dout, din] -> [2, P, NT, 512] (wT, dout-major blocks)."""
    wT = np.asarray(w).T
    return np.ascontiguousarray(
        wT.reshape(NT, P, 2, W).transpose(2, 1, 0, 3)).astype(np.float32)


def _dout_vec_pack(b):
    """[D] per-dout vector -> [P, NT]."""
    return np.ascontiguousarray(np.asarray(b).reshape(NT, P).T).astype(
        np.float32)


def _flags(inp):
    dec_mask = inp["dec_mask"]
    enc_mask = inp["enc_mask"]
    if not (np.all(dec_mask == 1.0) and np.all(enc_mask == 1.0)):
        raise NotImplementedError("padding masks not supported")
    ln_names = ["n1_w", "n1_b", "n2_w", "n2_b", "out_ln_w", "out_ln_b"]
    with_ln_wb = not all(
        np.all(inp[n] == (1.0 if n.endswith("w") else 0.0)) for n in ln_names)
    b_names = ["sa_qb", "sa_kb", "sa_vb", "out_b", "ca_qb", "ca_kb", "ca_vb"]
    with_bias = any(np.any(inp[n] != 0.0) for n in b_names)
    return with_ln_wb, with_bias


def build_in_maps(inputs):
    inp = {k: np.asarray(v) for k, v in inputs.items()}
    with_ln_wb, with_bias = _flags(inp)

    # weights (shared by all cores)
    bf = ml_dtypes.bfloat16
    wmap = {
        "wq": _w_pack(inp["sa_qw"]),
        "wk": _w_pack(inp["sa_kw"]).astype(bf),
        "wqc": _w_pack(inp["ca_qw"]),
        "wkc": _w_pack(inp["ca_kw"]).astype(bf),
        "wo": _w_pack(inp["out_w"]),
        "wv": _wv_pack(inp["sa_vw"]).astype(bf),
        "wvc": _wv_pack(inp["ca_vw"]).astype(bf),
    }
    if with_ln_wb:
        wmap["lnp"] = np.stack([_dout_vec_pack(inp[n]) for n in ln_names])
    if with_bias:
        # order: sa_qb, sa_kb, (slot 2 unused), out_b, ca_qb, ca_kb, (6 unused)
        wmap["bias"] = np.stack(
            [_dout_vec_pack(inp[n]) for n in
             ["sa_qb", "sa_kb", "sa_vb", "out_b", "ca_qb", "ca_kb", "ca_vb"]])
        wmap["vbflat"] = np.stack(
            [inp["sa_vb"].reshape(1, D).astype(np.float32),
             inp["ca_vb"].reshape(1, D).astype(np.float32)])

    # full-sequence bf16 feature-major packs per batch element
    xbf_b = [np.ascontiguousarray(
        inp["dec_hidden_states"][b].T.reshape(NT, P, LT).transpose(1, 0, 2)
    ).astype(ml_dtypes.bfloat16) for b in range(N)]
    ebf_b = [np.ascontiguousarray(
        inp["enc_outputs"][b].T.reshape(NT, P, LT).transpose(1, 0, 2)
    ).astype(ml_dtypes.bfloat16) for b in range(N)]

    in_maps = []
    for c in range(8):
        b, r = c // 2, c % 2
        cols = _role_cols(r)
        x_loc = _feat_pack(inp["dec_hidden_states"][b].T[:, cols])
        # diagonal-tile mask (same for every q-block): valid iff
        # p <= 64*r + j
        m = (np.arange(P)[:, None] <= 64 * r + np.arange(64)[None, :])
        m = np.ascontiguousarray(m).astype(ml_dtypes.bfloat16)
        im = {"x": x_loc, "xbf": xbf_b[b], "encbf": ebf_b[b], "masks": m}
        im.update(wmap)
        in_maps.append(im)
    return in_maps


def _role_cols(r):
    return np.concatenate(
        [np.arange(128 * j + 64 * r, 128 * j + 64 * r + 64) for j in range(8)])


def kernel(**inputs):
    inp = {k: np.asarray(v) for k, v in inputs.items()}
    key = _flags(inp)
    if key not in _CACHE:
        _CACHE[key] = _build_nc(*key)
    nc = _CACHE[key]
    in_maps = build_in_maps(inp)

    global LAST_RESULT
    res = run_bass_kernel_spmd(nc, in_maps, list(range(8)))
    LAST_RESULT = res

    out = np.zeros((N, LT, D), dtype=np.float32)
    for c in range(8):
        b, r = c // 2, c % 2
        y = res.results[c]["y"]  # [P, NT, W]
        out[b, _role_cols(r), :] = (
            np.asarray(y).transpose(1, 0, 2).reshape(D, W).T)
    return out


if __name__ == "__main__":
    _build_nc(False, False)
    print("built ok")

